# revision 2
# baseline (speedup 1.0000x reference)
import numpy as np

# Self-contained kernel for nn_CorrBlock2 on 8 NeuronCores.
# Strategy: shard n_p (points) across 8 devices with shard_map; the only
# cross-point coupling (GroupNorm statistics) is done with jax.lax.psum.
#
# Key algebraic simplifications (exact, verified against the reference):
#  * ids=3, num_iters=12 -> adaptive branch taken; the per-level length
#    factors (r1_len/r2_len magnitudes, incl. r1_max/r2_max) cancel inside
#    round(sum(diff*v)/||v||), so d0/d1 are IDENTICAL for all 3 levels.
#    => one 9-bin histogram; w1 folds to W1r = sum of its three 9-col blocks.
#  * (u1, u2) is an orthonormal frame => no r1_max/r2_max all-reduce needed.
#  * dynamic_k = 24 - 2*min(3,10) = 18.

_RES = 3
_NUM_LEVELS = 3
_KNN = 24
_EPS = 1e-8

_N_CORES = 8

_compiled = None


def _build():
    global _compiled
    if _compiled is not None:
        return _compiled

    import jax
    import jax.numpy as jnp
    from jax.sharding import Mesh, PartitionSpec as P
    from jax.experimental.shard_map import shard_map
    from functools import partial

    devs = jax.devices()[:_N_CORES]
    mesh = Mesh(np.array(devs), ("p",))

    def shard_fn(coords, f1, f2, corr, xy2,
                 w1r, b1, g1, beta1, a1, w2, b2,
                 wk, bk, gk, betak, ak, wo, bo):
        # coords [b,S,2] f1,f2 [b,S,2] corr [b,S,K] xy2 [b,S,K,2]
        b, S, K = corr.shape
        n_p_global = S * _N_CORES

        r1n = jnp.sqrt(jnp.sum(f1 * f1, -1, keepdims=True))          # [b,S,1]
        f2n = jnp.sqrt(jnp.sum(f2 * f2, -1, keepdims=True))
        u1 = f1 / r1n
        cos1 = jnp.sum(f1 * f2, -1, keepdims=True) / (
            jnp.maximum(r1n, _EPS) * jnp.maximum(f2n, _EPS))
        r2p = f2 - cos1 * f2n * u1
        r2n = jnp.sqrt(jnp.sum(r2p * r2p, -1, keepdims=True))
        u2 = r2p / r2n

        diff = xy2 - coords[:, :, None, :]                           # [b,S,K,2]
        w1c = diff[..., 0] * u1[..., 0:1] + diff[..., 1] * u1[..., 1:2]  # [b,S,K]
        w2c = diff[..., 0] * u2[..., 0:1] + diff[..., 1] * u2[..., 1:2]

        d0 = jnp.round(w1c)
        d1 = jnp.round(w2c)
        valid = (jnp.abs(d0) <= 1.0) & (jnp.abs(d1) <= 1.0)
        cube = jnp.where(valid, (d0 + 1.0) * 3.0 + (d1 + 1.0), 9.0)
        onehot = (cube[..., None] == jnp.arange(9, dtype=cube.dtype)
                  ).astype(corr.dtype)                               # [b,S,K,9]
        h = jnp.einsum("bskj,bsk->bjs", onehot, corr)                # [b,9,S]
        cnt = jnp.clip(jnp.sum(onehot, axis=2), 1.0, float(n_p_global))
        feats = h / cnt.transpose(0, 2, 1)                           # [b,9,S]

        x = jnp.einsum("oc,bcs->bos", w1r, feats) + b1[None, :, None]  # [b,64,S]

        # knn branch (pre-normalization part, before the single collective)
        dist = w1c * w1c + w2c * w2c                                 # [b,S,K]
        _, nbrs = jax.lax.top_k(-dist, 18)                           # [b,S,18]
        kc = jnp.take_along_axis(corr, nbrs, axis=2)                 # [b,S,18]
        kxy = jnp.take_along_axis(xy2, nbrs[..., None], axis=2)      # [b,S,18,2]
        kdx = kxy[..., 0] - coords[..., 0:1]
        kdy = kxy[..., 1] - coords[..., 1:2]
        yfeat = jnp.stack([kc, kdx, kdy], axis=-1)                   # [b,S,18,3]
        yh = jnp.einsum("of,bskf->bsko", wk, yfeat) + bk             # [b,S,18,64]

        # GroupNorm stats for BOTH branches via ONE all-reduce.
        xs = jnp.sum(x, axis=2)                                      # [b,64]
        xs2 = jnp.sum(x * x, axis=2)
        ys = jnp.sum(yh, axis=(1, 2))                                # [b,64]
        ys2 = jnp.sum(yh * yh, axis=(1, 2))
        stats = jax.lax.psum(jnp.stack([xs, xs2, ys, ys2], 0), "p")  # [4,b,64]
        xs, xs2, ys, ys2 = stats[0], stats[1], stats[2], stats[3]

        cnt_x = 8.0 * n_p_global
        mu_g = jnp.sum(xs.reshape(b, 8, 8), -1) / cnt_x              # [b,8]
        var_g = jnp.sum(xs2.reshape(b, 8, 8), -1) / cnt_x - mu_g * mu_g
        inv_g = jax.lax.rsqrt(var_g + 1e-5)
        mu_c = jnp.repeat(mu_g, 8, axis=1)[:, :, None]
        inv_c = jnp.repeat(inv_g, 8, axis=1)[:, :, None]
        xn = (x - mu_c) * inv_c * g1[None, :, None] + beta1[None, :, None]
        xh = jnp.where(xn >= 0, xn, a1 * xn)
        vox = jnp.einsum("oc,bcs->bos", w2, xh) + b2[None, :, None]  # [b,64,S]

        cnt_y = 8.0 * n_p_global * 18.0
        muy_g = jnp.sum(ys.reshape(b, 8, 8), -1) / cnt_y
        vary_g = jnp.sum(ys2.reshape(b, 8, 8), -1) / cnt_y - muy_g * muy_g
        invy_g = jax.lax.rsqrt(vary_g + 1e-5)
        muy_c = jnp.repeat(muy_g, 8, axis=1)[:, None, None, :]
        invy_c = jnp.repeat(invy_g, 8, axis=1)[:, None, None, :]
        yn = (yh - muy_c) * invy_c * gk[None, None, None, :] + betak[None, None, None, :]
        yp = jnp.where(yn >= 0, yn, ak * yn)
        ymax = jnp.max(yp, axis=2)                                   # [b,S,64]
        out = vox + jnp.einsum("oc,bsc->bos", wo, ymax) + bo[None, :, None]
        return out                                                   # [b,64,S]

    pt = P(None, "p")
    ptk = P(None, "p", None)
    ptk2 = P(None, "p", None, None)
    rep = P()
    fn = shard_map(
        shard_fn, mesh=mesh,
        in_specs=(pt, pt, pt, ptk, ptk2,
                  rep, rep, rep, rep, rep, rep, rep,
                  rep, rep, rep, rep, rep, rep, rep),
        out_specs=P(None, None, "p"),
        check_rep=False,
    )
    jfn = jax.jit(fn)
    _compiled = (jax, jnp, mesh, jfn)
    return _compiled


def bench(inputs, N=40):
    import time
    jax, jnp, mesh, jfn = _build()
    from jax.sharding import NamedSharding, PartitionSpec as P
    import numpy as np
    w1 = inputs['w1'].astype(np.float32)
    w1r = w1[:, :9] + w1[:, 9:18] + w1[:, 18:27]
    args = (inputs['coords'].astype(np.float32),
            inputs['all_delta_flow'][-1].astype(np.float32),
            inputs['all_delta_flow'][-2].astype(np.float32),
            inputs['truncated_corr'].astype(np.float32),
            inputs['truncate_xy2'].astype(np.float32),
            w1r, inputs['b1'].astype(np.float32), inputs['g1'].astype(np.float32),
            inputs['beta1'].astype(np.float32), np.float32(inputs['a1'][0]),
            inputs['w2'].astype(np.float32), inputs['b2'].astype(np.float32),
            inputs['wk'].astype(np.float32), inputs['bk'].astype(np.float32),
            inputs['gk'].astype(np.float32), inputs['betak'].astype(np.float32),
            np.float32(inputs['ak'][0]),
            inputs['wo'].astype(np.float32), inputs['bo'].astype(np.float32))
    pt = P(None, 'p')
    specs = (pt, pt, pt, P(None, 'p', None), P(None, 'p', None, None)) + (P(),) * 14
    dargs = [jax.device_put(a, NamedSharding(mesh, s)) for a, s in zip(args, specs)]
    out = jfn(*dargs)
    jax.block_until_ready(out)
    t0 = time.perf_counter()
    for _ in range(N):
        out = jfn(*dargs)
    jax.block_until_ready(out)
    return (time.perf_counter() - t0) / N


def kernel(coords, all_delta_flow, truncated_corr, truncate_xy2,
           w1, b1, g1, beta1, a1, w2, b2,
           wk, bk, gk, betak, ak, wo, bo, num_iters, scale):
    jax, jnp, mesh, jfn = _build()

    ids = all_delta_flow.shape[0]
    ni = int(np.asarray(num_iters))
    adaptive = (ids >= 2) and (ids < ni - 2)
    assert adaptive, "kernel specialized for the adaptive branch (ids=3, num_iters=12)"
    dk = _KNN - 2 * min(ids, 10)
    assert dk == 18

    coords = np.asarray(coords, np.float32)
    f1 = np.asarray(all_delta_flow[-1], np.float32)
    f2 = np.asarray(all_delta_flow[-2], np.float32)
    corr = np.asarray(truncated_corr, np.float32)
    xy2 = np.asarray(truncate_xy2, np.float32)

    # fold the 3 identical levels into one [64,9] matrix
    w1 = np.asarray(w1, np.float32)
    w1r = w1[:, 0:9] + w1[:, 9:18] + w1[:, 18:27]

    args = (coords, f1, f2, corr, xy2,
            w1r, np.asarray(b1, np.float32), np.asarray(g1, np.float32),
            np.asarray(beta1, np.float32), np.asarray(a1, np.float32).reshape(()),
            np.asarray(w2, np.float32), np.asarray(b2, np.float32),
            np.asarray(wk, np.float32), np.asarray(bk, np.float32),
            np.asarray(gk, np.float32), np.asarray(betak, np.float32),
            np.asarray(ak, np.float32).reshape(()),
            np.asarray(wo, np.float32), np.asarray(bo, np.float32))
    out = jfn(*args)
    return np.asarray(jax.block_until_ready(out))

